# revision 25
# baseline (speedup 1.0000x reference)
"""DCGRU cell on 8 Trainium2 NeuronCores.

Sharding: data-parallel over batch (B=32 -> 4 per core), adjacency + MLP
weights replicated. No collectives; host gathers per-core outputs.

Per-core layouts (all bf16 matmul datapath, f32 accum):
  x node-major:  [16 tiles][128 nodes, 256] cols = b*64+fx    (diffusion lhsT)
  h node-major:  [16 tiles][128 nodes, 512] cols = b*128+fh
  x feat-major:  [2 tiles][128 rows=b*64+fx, 2048 nodes]      (hop outputs, MLP rhs)
  h feat-major:  [4 tiles][128 rows=b*128+fh, 2048 nodes]
Hop matmul: out_fm[bf, i] = sum_j x_nm[j, bf] * W[i, j]
  = matmul(lhsT=nm[jt][:, c*128:+128], rhs=WT[jt][:, i-block]) accumulated
  over jt in PSUM; W streamed host-pretransposed (WT[j, i] = W[i, j]).
x/h segregation pays off twice:
  - diffusion 2 hops only r*h (the x columns of [x | r*h] are unchanged
    from diffusion 1); diff-1 x-hop outputs are spilled to DRAM and
    reloaded for diffusion-2 MLP feeds.
  - MLP per (b, n-block) is one K=128 matmul (h segment, full tile) plus
    one K=64 matmul (x segment); the K=64s of a batch pair sit at row
    offsets 0/64 and run concurrently on disjoint PE row groups.
MLP: gate logits acc[b][o, n] accumulated across hops in DRAM via
  accum_op=add DMAs straight from PSUM staging.
Chain re-entry: fm -> nm via PE transposes (hops 1,2 of each direction).
"""

import sys
import numpy as np
import ml_dtypes

for _p in ("/opt/trn_rl_repo",):
    if _p not in sys.path:
        sys.path.insert(0, _p)

from concourse import bacc, tile, mybir  # noqa: E402
from concourse.alu_op_type import AluOpType as ALU  # noqa: E402
from concourse.bass_utils import run_bass_kernel_spmd  # noqa: E402

F32 = mybir.dt.float32
BF16 = mybir.dt.bfloat16
F8 = mybir.dt.float8e4
MMDT = BF16
AF = mybir.ActivationFunctionType
DR = mybir.MatmulPerfMode.DoubleRow
SW = 512.0          # fp8 scale on W and on k>=1 gate weights
SA = 32.0           # fp8 scale on chain activations
SPS = SW * SA       # psum scale of fp8 matmuls

C = 4          # batches per core
DX = 64        # x features per batch
DH = 128       # h features per batch
BFX = C * DX   # 256
BFH = C * DH   # 512
NCORES = 8
NHOPS = 3


def build_nc(nt=16):
    """Build + compile the per-core Bass kernel. nt = node tiles (N = nt*128)."""
    N = nt * 128
    nbk = N // 512
    NXT = BFX // 128   # 2 x fm tiles
    NHT = BFH // 128   # 4 h fm tiles

    nc = bacc.Bacc("TRN2", target_bir_lowering=False, debug=False,
                   num_devices=NCORES)

    def din(name, shape, dt=F32):
        return nc.dram_tensor(name, shape, dt, kind="ExternalInput").ap()

    njp = nt // 2
    XNM = din("x_nm", [njp, 128, 2, BFX], F8)
    HNM = din("h_nm", [njp, 128, 2, BFH], F8)
    XFM0 = din("x_fm0", [NXT, 128, N], MMDT)
    HFM0 = din("h_fm0", [NHT, 128, N], MMDT)
    HFMF = din("h_fmf", [C, 128, N])
    WFT = din("wfT", [njp, 128, 2, N], F8)
    WBT = din("wbT", [njp, 128, 2, N], F8)
    WRX = din("wrx", [128, 256], MMDT)
    WRH = din("wrh", [128, 128], MMDT)
    WZX = din("wzx", [128, 256], MMDT)
    WZH = din("wzh", [128, 128], MMDT)
    WNX = din("wnx", [128, 256], MMDT)
    WNH = din("wnh", [128, 128], MMDT)
    WRX8 = din("wrx8", [6, 128, 256], F8)
    WRH8 = din("wrh8", [6, 128, 128], F8)
    WZX8 = din("wzx8", [6, 128, 256], F8)
    WZH8 = din("wzh8", [6, 128, 128], F8)
    WNX8 = din("wnx8", [6, 128, 256], F8)
    WNH8 = din("wnh8", [6, 128, 128], F8)
    BR = din("br_c", [128, 1])
    BZ = din("bz_c", [128, 1])
    BN = din("bn_c", [128, 1])
    IDT = din("ident", [128, 128], F8)
    OUT = nc.dram_tensor("out_fm", [C, 128, N], F32, kind="ExternalOutput").ap()

    XFMS = nc.dram_tensor("xfm_sp", [2 * NHOPS, NXT, 128, N], F8).ap()
    RHNM = nc.dram_tensor("rh_nm_d", [njp, 128, 2, BFH], F8).ap()

    with tile.TileContext(nc) as tc:
        with (
            tc.tile_pool(name="xnm", bufs=16) as xnm_pool,
            tc.tile_pool(name="hnm", bufs=16) as hnm_pool,
            tc.tile_pool(name="xfm", bufs=6) as xfm_pool,
            tc.tile_pool(name="xfmb", bufs=3) as xfmb_pool,
            tc.tile_pool(name="hfm", bufs=9) as hfm_pool,
            tc.tile_pool(name="hfmb", bufs=5) as hfmb_pool,
            tc.tile_pool(name="acc", bufs=16) as acc_pool,
            tc.tile_pool(name="gate", bufs=5) as gate_pool,
            tc.tile_pool(name="wt", bufs=12) as wt_pool,
            tc.tile_pool(name="wxi", bufs=8) as wxi_pool,
            tc.tile_pool(name="const", bufs=1) as const_pool,
            tc.tile_pool(name="ps", bufs=4, space="PSUM") as ps_pool,
            tc.tile_pool(name="psx", bufs=4, space="PSUM") as psx_pool,
        ):
            ident = const_pool.tile([128, 128], F8, tag="ident")
            nc.sync.dma_start(ident[:], IDT[:])
            brt = const_pool.tile([128, 1], F32, tag="brt")
            nc.sync.dma_start(brt[:], BR[:])
            bzt = const_pool.tile([128, 1], F32, tag="bzt")
            nc.sync.dma_start(bzt[:], BZ[:])
            bnt = const_pool.tile([128, 1], F32, tag="bnt")
            nc.sync.dma_start(bnt[:], BN[:])

            def load_xnm(eng=None):
                eng = eng or nc.sync
                ts = []
                for jp in range(njp):
                    t = xnm_pool.tile([128, 2, BFX], F8, name="xnmt", tag="xnm")
                    eng.dma_start(t[:], XNM[jp])
                    ts.append(t)
                return ts

            def load_hnm(SRC, eng=None):
                eng = eng or nc.sync
                ts = []
                for jp in range(njp):
                    t = hnm_pool.tile([128, 2, BFH], F8, name="hnmt", tag="hnm")
                    eng.dma_start(t[:], SRC[jp])
                    ts.append(t)
                return ts

            def hop(src_x, src_h, WT, transpose_out=False):
                """One diffusion hop; x part optional.
                Returns (fx, fh, nxs, nhs): fm output tiles and (if
                transpose_out) the re-transposed nm tiles for the next hop.
                c-major: one full jt-accumulation run per output slice, then
                drain that PSUM bank -- only ~3 banks live at a time, so the
                next run never stalls on bank recycling. Chain re-entry
                transposes are folded in per-ibk so the next hop never waits
                on a bulk transpose pass."""
                with_x = src_x is not None
                fx = ([xfm_pool.tile([128, N], F8, name="xfmt", tag="xfm")
                       for _ in range(NXT)] if with_x else None)
                fh = [hfm_pool.tile([128, N], F8, name="hfmt", tag="hfm")
                      for _ in range(NHT)]
                nxs = nhs = None
                if transpose_out:
                    if with_x:
                        nxs = [xnm_pool.tile([128, 2, BFX], F8, name="xnmt",
                                             tag="xnm") for _ in range(njp)]
                    nhs = [hnm_pool.tile([128, 2, BFH], F8, name="hnmt",
                                         tag="hnm") for _ in range(njp)]

                def transpose_blk(ibk):
                    wid = (BFX if with_x else 0) + BFH
                    for q in range(4):
                        it = 4 * ibk + q
                        jp, pl = divmod(it, 2)
                        its = slice(128 * it, 128 * (it + 1))
                        # fp8 transpose-mode PSUM writes need element step 2
                        pt = ps_pool.tile([128, wid, 2], F8,
                                          name="pst", tag="ps")
                        off = 0
                        if with_x:
                            for c in range(NXT):
                                nc.tensor.transpose(
                                    pt[:, 128 * c:128 * (c + 1), 0],
                                    fx[c][:, its], ident[:])
                            off = BFX
                        for c in range(NHT):
                            nc.tensor.transpose(
                                pt[:, off + 128 * c:off + 128 * (c + 1), 0],
                                fh[c][:, its], ident[:])
                        if with_x:
                            nc.vector.tensor_copy(nxs[jp][:, pl, :],
                                                  pt[:, 0:BFX, 0])
                        nc.vector.tensor_copy(nhs[jp][:, pl, :],
                                              pt[:, off:off + BFH, 0])

                # W in [128,2,1024] half-row tiles: each (jp, c) stationary
                # feeds TWO 512-col matmuls, so the 256-col LDWEIGHTS (~213ns,
                # no FWL in DoubleRow mode) hides behind ~214ns of streaming.
                nhh = nbk // 2
                for hh in range(nhh):
                    hbs = slice(1024 * hh, 1024 * (hh + 1))
                    wts = []
                    for jp in range(njp):
                        wt = wt_pool.tile([128, 2, 1024], F8, name="wtt",
                                          tag="wt")
                        nc.sync.dma_start(wt[:], WT[jp][:, :, hbs])
                        wts.append(wt)
                    runs = []
                    if with_x:
                        runs += [(src_x, c, fx[c]) for c in range(NXT)]
                    runs += [(src_h, c, fh[c]) for c in range(NHT)]
                    first_run = True
                    for src, c, fm in runs:
                        p0 = ps_pool.tile([128, 512], F32, name="pst", tag="ps")
                        p1 = ps_pool.tile([128, 512], F32, name="pst", tag="ps")
                        for jp in range(njp):
                            st = src[jp][:, :, 128 * c:128 * (c + 1)]
                            nc.tensor.matmul(
                                p0[:], st, wts[jp][:, :, 0:512],
                                start=(jp == 0), stop=(jp == njp - 1),
                                perf_mode=DR)
                            nc.tensor.matmul(
                                p1[:], st, wts[jp][:, :, 512:1024],
                                start=(jp == 0), stop=(jp == njp - 1),
                                perf_mode=DR)
                        nc.vector.tensor_scalar_mul(
                            fm[:, 1024 * hh:1024 * hh + 512], p0[:], 1.0 / SW)
                        nc.vector.tensor_scalar_mul(
                            fm[:, 1024 * hh + 512:1024 * (hh + 1)], p1[:],
                            1.0 / SW)
                        # previous half's transposes go after this half's
                        # first run so the PE never waits on fm-copy drains.
                        if transpose_out and hh > 0 and first_run:
                            for ib in (2 * hh - 2, 2 * hh - 1):
                                transpose_blk(ib)
                            first_run = False
                if transpose_out:
                    transpose_blk(nbk - 2)
                    transpose_blk(nbk - 1)
                return fx, fh, nxs, nhs

            def mlp_feed(fx, fh, kidx, gates, first):
                """gates: list of (WXbf, WHbf, WX8, WH8, acc_tiles).
                acc_tiles: 2*C SBUF tiles [128, NH] holding SPS-scaled
                logit partial sums; every feed accumulates in SBUF (no
                DRAM round-trip). kidx==0 feeds are bf16 at true scale
                (scaled up by SPS on write); kidx>=1 feeds are fp8 with
                SPS-scaled PSUM."""
                NHl = N // 2
                for WXB, WHB, WX8, WH8, acc in gates:
                    if kidx == 0:
                        wx = wxi_pool.tile([128, 256], MMDT, name="wxt", tag="wxi")
                        nc.scalar.dma_start(wx[:], WXB[:])
                        wh = wxi_pool.tile([128, 128], MMDT, name="wht", tag="wxi")
                        nc.scalar.dma_start(wh[:], WHB[:])
                    else:
                        wx = wxi_pool.tile([128, 256], F8, name="wxt", tag="wxi")
                        nc.scalar.dma_start(wx[:], WX8[kidx - 1])
                        wh = wxi_pool.tile([128, 128], F8, name="wht", tag="wxi")
                        nc.scalar.dma_start(wh[:], WH8[kidx - 1])
                    # b-outer: each batch's logits finish (and downstream
                    # gate work can start) before the next batch's matmuls.
                    w = min(NHl, 512)
                    for b in range(C):
                        pss = [psx_pool.tile([128, 512], F32, name="psxt",
                                             tag="psx") for _ in range(nbk)]
                        for nb in range(nbk):
                            nc.tensor.matmul(
                                pss[nb][:], wh[:],
                                fh[b][:, 512 * nb:512 * (nb + 1)],
                                start=True, stop=False)
                        wxs = wx[:, 128 * (b % 2):128 * (b % 2 + 1)]
                        for nb in range(nbk):
                            nc.tensor.matmul(
                                pss[nb][:], wxs,
                                fx[b // 2][:, 512 * nb:512 * (nb + 1)],
                                start=False, stop=True)
                        for nb in range(nbk):
                            for sub in range(max(1, 512 // NHl)):
                                col = 512 * nb + w * sub
                                pt = acc[2 * b + col // NHl]
                                off = col % NHl
                                pv = pss[nb][:, w * sub:w * (sub + 1)]
                                if first:
                                    nc.vector.tensor_scalar_mul(
                                        pt[:, off:off + w], pv, SPS)
                                else:
                                    nc.vector.tensor_add(
                                        pt[:, off:off + w], pv,
                                        pt[:, off:off + w])

            def load_xfm_spill(kidx):
                ts = []
                for c in range(NXT):
                    t = xfm_pool.tile([128, N], F8, name="xfmt", tag="xfm")
                    nc.scalar.dma_start(t[:], XFMS[kidx - 1][c])
                    ts.append(t)
                return ts

            # ---------------- diffusion 1 (r, z gates) ----------------
            NH = N // 2
            racc = [acc_pool.tile([128, NH], F32, name="acct", tag="acc")
                    for _ in range(2 * C)]
            zacc = [acc_pool.tile([128, NH], F32, name="acct", tag="acc")
                    for _ in range(2 * C)]
            gates1 = [(WRX, WRH, WRX8, WRH8, racc), (WZX, WZH, WZX8, WZH8, zacc)]
            xfm0 = []
            for c in range(NXT):
                t = xfmb_pool.tile([128, N], MMDT, name="xfmbt", tag="xfmb")
                nc.scalar.dma_start(t[:], XFM0[c])
                xfm0.append(t)
            hfm0 = []
            for c in range(NHT):
                t = hfmb_pool.tile([128, N], MMDT, name="hfmbt", tag="hfmb")
                nc.scalar.dma_start(t[:], HFM0[c])
                hfm0.append(t)

            pending = (xfm0, hfm0, 0, gates1, True)
            cur_x, cur_h = load_xnm(), load_hnm(HNM, nc.scalar)
            nxt_nm = None
            for wdir, WT in ((0, WFT), (1, WBT)):
                if wdir == 1:
                    cur_x, cur_h = nxt_nm
                for k in range(1, NHOPS + 1):
                    if k == NHOPS and wdir == 0:
                        # prefetch the backward-chain inputs under this hop
                        nxt_nm = (load_xnm(nc.scalar), load_hnm(HNM, nc.scalar))
                    fx, fh, cur_x, cur_h = hop(cur_x, cur_h, WT,
                                               transpose_out=(k < NHOPS))
                    kidx = wdir * NHOPS + k
                    for c in range(NXT):
                        nc.scalar.dma_start(XFMS[kidx - 1][c], fx[c][:])
                    if pending is not None:
                        mlp_feed(*pending)
                    pending = (fx, fh, kidx, gates1, False)
            mlp_feed(*pending)
            # z logits are final here -- run the sigmoids now, under
            # diffusion 2's compute, so the tail only runs tanh.
            for i in range(2 * C):
                nc.scalar.activation(zacc[i][:], zacc[i][:], AF.Sigmoid,
                                     bias=bzt[:], scale=1.0 / SPS)

            # ------------- gates r, z; assemble rh (nm + fm) -------------
            rh_fm = []
            rh_nm = [hnm_pool.tile([128, 2, BFH], F8, name="hnmt", tag="hnm")
                     for _ in range(njp)]
            for b in range(C):
                rh = hfmb_pool.tile([128, N], MMDT, name="hfmbt", tag="hfmb")
                rh8 = hfm_pool.tile([128, N], F8, name="hfmt", tag="hfm")
                for hx in range(2):
                    hs = slice(NH * hx, NH * (hx + 1))
                    accr = racc[2 * b + hx]
                    nc.scalar.activation(accr[:], accr[:], AF.Sigmoid,
                                         bias=brt[:], scale=1.0 / SPS)
                    h = gate_pool.tile([128, NH], F32, name="gatet", tag="gate")
                    nc.sync.dma_start(h[:], HFMF[b][:, hs])
                    nc.vector.tensor_mul(rh[:, hs], accr[:], h[:])
                    nc.vector.tensor_scalar_mul(rh8[:, hs], rh[:, hs], SA)
                rh_fm.append(rh)
                # rh columns of rh_nm (PE transpose 128-blocks, fp8 planes)
                for g in range(nt // 4):
                    ps = psx_pool.tile([128, 512, 2], F8, name="psxt", tag="psx")
                    for q in range(4):
                        it = 4 * g + q
                        nc.tensor.transpose(
                            ps[:, 128 * q:128 * (q + 1), 0],
                            rh8[:, 128 * it:128 * (it + 1)], ident[:])
                    for q in range(4):
                        it = 4 * g + q
                        jp, pl = divmod(it, 2)
                        nc.vector.tensor_copy(
                            rh_nm[jp][:, pl, b * DH:(b + 1) * DH],
                            ps[:, 128 * q:128 * (q + 1), 0])
            # spill rh_nm for the backward-chain reload
            for jp in range(njp):
                nc.scalar.dma_start(RHNM[jp], rh_nm[jp][:])

            # ---------------- diffusion 2 (n gate) ----------------
            nacc = [acc_pool.tile([128, NH], F32, name="acct", tag="acc")
                    for _ in range(2 * C)]
            gates2 = [(WNX, WNH, WNX8, WNH8, nacc)]
            xfm0b = []
            for c in range(NXT):
                t = xfmb_pool.tile([128, N], MMDT, name="xfmbt", tag="xfmb")
                nc.scalar.dma_start(t[:], XFM0[c])
                xfm0b.append(t)
            pending = (xfm0b, rh_fm, 0, gates2, True)
            cur_h = rh_nm
            nxt_h = None
            for wdir, WT in ((0, WFT), (1, WBT)):
                if wdir == 1:
                    cur_h = nxt_h
                for k in range(1, NHOPS + 1):
                    if k == NHOPS and wdir == 0:
                        nxt_h = load_hnm(RHNM, nc.scalar)
                    _, fh, _, cur_h = hop(None, cur_h, WT,
                                          transpose_out=(k < NHOPS))
                    kidx = wdir * NHOPS + k
                    if pending is not None:
                        mlp_feed(*pending)
                    pending = (load_xfm_spill(kidx), fh, kidx, gates2, False)
            mlp_feed(*pending)

            # ---------------- final gate ----------------
            # out = h + z*(tanh(acc_n) - h); z and n logits live in SBUF,
            # chunked + in-place so chunks pipeline across ACT / DVE / DMA.
            for b in range(C):
                for hx in range(2):
                    hs = slice(NH * hx, NH * (hx + 1))
                    accn = nacc[2 * b + hx]
                    nc.scalar.activation(accn[:], accn[:], AF.Tanh,
                                         bias=bnt[:], scale=1.0 / SPS)
                    h = gate_pool.tile([128, NH], F32, name="gatet", tag="gate")
                    nc.sync.dma_start(h[:], HFMF[b][:, hs])
                    accz = zacc[2 * b + hx]
                    nc.vector.tensor_sub(accn[:], accn[:], h[:])
                    if (2 * b + hx) % 2 == 0:
                        nc.vector.tensor_mul(accz[:], accz[:], accn[:])
                        nc.vector.tensor_add(h[:], accz[:], h[:])
                    else:
                        # alternate chunks blend on the otherwise-idle GpSimd
                        # so the tail's elementwise work runs two-wide
                        nc.gpsimd.tensor_mul(accz[:], accz[:], accn[:])
                        nc.gpsimd.tensor_add(h[:], accz[:], h[:])
                    nc.sync.dma_start(OUT[b][:, hs], h[:])

    nc.compile()
    return nc


def _pack_gate(W):
    """[128, 7*192] torch-Linear weight -> bf16 k=0 pack (WXB [128,256]
    zero-padded even/odd, WHB [128,128]) + fp8 SW-scaled packs for k=1..6
    (WX8 [6,128,256], WH8 [6,128,128])."""
    f8 = np.dtype(ml_dtypes.float8_e4m3)
    bf = np.dtype(ml_dtypes.bfloat16)
    def xpack(k):
        xs = W[:, k * 192:k * 192 + DX].T          # [64,128]
        out = np.zeros((128, 256), np.float32)
        out[0:64, 0:128] = xs
        out[64:128, 128:256] = xs
        return out
    def hpack(k):
        return W[:, k * 192 + DX:(k + 1) * 192].T  # [128,128]
    wxb = np.ascontiguousarray(xpack(0)).astype(bf)
    whb = np.ascontiguousarray(hpack(0)).astype(bf)
    wx8 = np.ascontiguousarray(np.clip(
        np.stack([xpack(k) * SW_H for k in range(1, 7)]), -240, 240)).astype(f8)
    wh8 = np.ascontiguousarray(np.clip(
        np.stack([hpack(k) * SW_H for k in range(1, 7)]), -240, 240)).astype(f8)
    return wxb, whb, wx8, wh8


SW_H = 512.0
SA_H = 32.0


def _dr_pack(nm_flat, width):
    """[N, width] node-major -> [N//256, 128, 2, width] DoubleRow pairs."""
    N = nm_flat.shape[0]
    t = nm_flat.reshape(N // 128, 128, width)           # [nt, p, w]
    t = t.reshape(N // 256, 2, 128, width).transpose(0, 2, 1, 3)
    return np.ascontiguousarray(t)


_NC_CACHE = {}


def _get_nc(nt):
    if nt not in _NC_CACHE:
        _NC_CACHE[nt] = build_nc(nt)
    return _NC_CACHE[nt]


def make_in_maps(x, h_prev, W_fwd, W_bwd, Wr, br, Wz, bz, Wn, bn):
    f8 = np.dtype(ml_dtypes.float8_e4m3)
    mdt = np.dtype(ml_dtypes.bfloat16)
    x = np.asarray(x, np.float32)
    h_prev = np.asarray(h_prev, np.float32)
    B, N, Din = x.shape
    nt = N // 128
    WfT = np.ascontiguousarray(np.asarray(W_fwd, np.float32).T)   # [N, N] (j, i)
    WbT = np.ascontiguousarray(np.asarray(W_bwd, np.float32).T)
    WfT8 = np.clip(_dr_pack(WfT * SW_H, N), -240, 240).astype(f8)
    WbT8 = np.clip(_dr_pack(WbT * SW_H, N), -240, 240).astype(f8)
    wrxb, wrhb, wrx8, wrh8 = _pack_gate(np.asarray(Wr, np.float32))
    wzxb, wzhb, wzx8, wzh8 = _pack_gate(np.asarray(Wz, np.float32))
    wnxb, wnhb, wnx8, wnh8 = _pack_gate(np.asarray(Wn, np.float32))
    ident = np.ascontiguousarray(np.eye(128, dtype=np.float32)).astype(f8)
    packs = dict(
        wrx=wrxb, wrh=wrhb, wrx8=wrx8, wrh8=wrh8,
        wzx=wzxb, wzh=wzhb, wzx8=wzx8, wzh8=wzh8,
        wnx=wnxb, wnh=wnhb, wnx8=wnx8, wnh8=wnh8)
    brc = np.ascontiguousarray(np.asarray(br, np.float32).reshape(128, 1))
    bzc = np.ascontiguousarray(np.asarray(bz, np.float32).reshape(128, 1))
    bnc = np.ascontiguousarray(np.asarray(bn, np.float32).reshape(128, 1))
    ncores = B // C
    in_maps = []
    for cix in range(ncores):
        xs = x[C * cix:C * (cix + 1)]          # [C,N,64]
        hs = h_prev[C * cix:C * (cix + 1)]     # [C,N,128]
        x_nm = np.ascontiguousarray(xs.transpose(1, 0, 2).reshape(N, BFX))
        h_nm = np.ascontiguousarray(hs.transpose(1, 0, 2).reshape(N, BFH))
        x_nm8 = np.clip(_dr_pack(x_nm * SA_H, BFX), -240, 240).astype(f8)
        h_nm8 = np.clip(_dr_pack(h_nm * SA_H, BFH), -240, 240).astype(f8)
        x_fm0 = np.ascontiguousarray(
            xs.transpose(0, 2, 1).reshape(BFX, N)).reshape(2, 128, N)
        h_fmf = np.ascontiguousarray(hs.transpose(0, 2, 1))  # [C,128,N]
        h_fm0 = h_fmf.reshape(4, 128, N)
        in_maps.append(dict(
            x_nm=x_nm8, h_nm=h_nm8,
            x_fm0=x_fm0.astype(mdt), h_fm0=h_fm0.astype(mdt),
            h_fmf=h_fmf,
            wfT=WfT8, wbT=WbT8, **packs,
            br_c=brc, bz_c=bzc, bn_c=bnc, ident=ident))
    return in_maps, nt, ncores


def kernel(x, h_prev, W_fwd, W_bwd, Wr, br, Wz, bz, Wn, bn, _trace=False):
    in_maps, nt, ncores = make_in_maps(
        x, h_prev, W_fwd, W_bwd, Wr, br, Wz, bz, Wn, bn)
    nc = _get_nc(nt)
    res = run_bass_kernel_spmd(nc, in_maps, list(range(ncores)), trace=_trace)
    outs = [np.ascontiguousarray(res.results[c]["out_fm"].transpose(0, 2, 1))
            for c in range(ncores)]
    full = np.concatenate(outs, axis=0).astype(np.float32)
    if _trace:
        return full, res
    return full


# revision 26
# speedup vs baseline: 1.0095x; 1.0095x over previous
"""DCGRU cell on 8 Trainium2 NeuronCores.

Sharding: data-parallel over batch (B=32 -> 4 per core), adjacency + MLP
weights replicated. No collectives; host gathers per-core outputs.

Per-core layouts (all bf16 matmul datapath, f32 accum):
  x node-major:  [16 tiles][128 nodes, 256] cols = b*64+fx    (diffusion lhsT)
  h node-major:  [16 tiles][128 nodes, 512] cols = b*128+fh
  x feat-major:  [2 tiles][128 rows=b*64+fx, 2048 nodes]      (hop outputs, MLP rhs)
  h feat-major:  [4 tiles][128 rows=b*128+fh, 2048 nodes]
Hop matmul: out_fm[bf, i] = sum_j x_nm[j, bf] * W[i, j]
  = matmul(lhsT=nm[jt][:, c*128:+128], rhs=WT[jt][:, i-block]) accumulated
  over jt in PSUM; W streamed host-pretransposed (WT[j, i] = W[i, j]).
x/h segregation pays off twice:
  - diffusion 2 hops only r*h (the x columns of [x | r*h] are unchanged
    from diffusion 1); diff-1 x-hop outputs are spilled to DRAM and
    reloaded for diffusion-2 MLP feeds.
  - MLP per (b, n-block) is one K=128 matmul (h segment, full tile) plus
    one K=64 matmul (x segment); the K=64s of a batch pair sit at row
    offsets 0/64 and run concurrently on disjoint PE row groups.
MLP: gate logits acc[b][o, n] accumulated across hops in DRAM via
  accum_op=add DMAs straight from PSUM staging.
Chain re-entry: fm -> nm via PE transposes (hops 1,2 of each direction).
"""

import sys
import numpy as np
import ml_dtypes

for _p in ("/opt/trn_rl_repo",):
    if _p not in sys.path:
        sys.path.insert(0, _p)

from concourse import bacc, tile, mybir  # noqa: E402
from concourse.alu_op_type import AluOpType as ALU  # noqa: E402
from concourse.bass_utils import run_bass_kernel_spmd  # noqa: E402

F32 = mybir.dt.float32
BF16 = mybir.dt.bfloat16
F8 = mybir.dt.float8e4
MMDT = BF16
AF = mybir.ActivationFunctionType
DR = mybir.MatmulPerfMode.DoubleRow
SW = 512.0          # fp8 scale on W and on k>=1 gate weights
SA = 32.0           # fp8 scale on chain activations
SPS = SW * SA       # psum scale of fp8 matmuls

C = 4          # batches per core
DX = 64        # x features per batch
DH = 128       # h features per batch
BFX = C * DX   # 256
BFH = C * DH   # 512
NCORES = 8
NHOPS = 3


def build_nc(nt=16):
    """Build + compile the per-core Bass kernel. nt = node tiles (N = nt*128)."""
    N = nt * 128
    nbk = N // 512
    NXT = BFX // 128   # 2 x fm tiles
    NHT = BFH // 128   # 4 h fm tiles

    nc = bacc.Bacc("TRN2", target_bir_lowering=False, debug=False,
                   num_devices=NCORES)

    def din(name, shape, dt=F32):
        return nc.dram_tensor(name, shape, dt, kind="ExternalInput").ap()

    njp = nt // 2
    XNM = din("x_nm", [njp, 128, 2, BFX], F8)
    HNM = din("h_nm", [njp, 128, 2, BFH], F8)
    XFM0 = din("x_fm0", [NXT, 128, N], MMDT)
    HFM0 = din("h_fm0", [NHT, 128, N], MMDT)
    HFMF = din("h_fmf", [C, 128, N])
    WFT = din("wfT", [njp, 128, 2, N], F8)
    WBT = din("wbT", [njp, 128, 2, N], F8)
    WRX = din("wrx", [128, 256], MMDT)
    WRH = din("wrh", [128, 128], MMDT)
    WZX = din("wzx", [128, 256], MMDT)
    WZH = din("wzh", [128, 128], MMDT)
    WNX = din("wnx", [128, 256], MMDT)
    WNH = din("wnh", [128, 128], MMDT)
    WRX8 = din("wrx8", [6, 128, 256], F8)
    WRH8 = din("wrh8", [6, 128, 128], F8)
    WZX8 = din("wzx8", [6, 128, 256], F8)
    WZH8 = din("wzh8", [6, 128, 128], F8)
    WNX8 = din("wnx8", [6, 128, 256], F8)
    WNH8 = din("wnh8", [6, 128, 128], F8)
    BR = din("br_c", [128, 1])
    BZ = din("bz_c", [128, 1])
    BN = din("bn_c", [128, 1])
    IDT = din("ident", [128, 128], F8)
    OUT = nc.dram_tensor("out_fm", [C, 128, N], F32, kind="ExternalOutput").ap()

    XFMS = nc.dram_tensor("xfm_sp", [2 * NHOPS, NXT, 128, N], F8).ap()
    RHNM = nc.dram_tensor("rh_nm_d", [njp, 128, 2, BFH], F8).ap()

    with tile.TileContext(nc) as tc:
        with (
            tc.tile_pool(name="xnm", bufs=16) as xnm_pool,
            tc.tile_pool(name="hnm", bufs=16) as hnm_pool,
            tc.tile_pool(name="xfm", bufs=6) as xfm_pool,
            tc.tile_pool(name="xfmb", bufs=3) as xfmb_pool,
            tc.tile_pool(name="hfm", bufs=9) as hfm_pool,
            tc.tile_pool(name="hfmb", bufs=5) as hfmb_pool,
            tc.tile_pool(name="acc", bufs=16) as acc_pool,
            tc.tile_pool(name="gate", bufs=5) as gate_pool,
            tc.tile_pool(name="wt", bufs=12) as wt_pool,
            tc.tile_pool(name="wxi", bufs=8) as wxi_pool,
            tc.tile_pool(name="const", bufs=1) as const_pool,
            tc.tile_pool(name="ps", bufs=4, space="PSUM") as ps_pool,
            tc.tile_pool(name="psx", bufs=4, space="PSUM") as psx_pool,
        ):
            ident = const_pool.tile([128, 128], F8, tag="ident")
            nc.sync.dma_start(ident[:], IDT[:])
            brt = const_pool.tile([128, 1], F32, tag="brt")
            nc.sync.dma_start(brt[:], BR[:])
            bzt = const_pool.tile([128, 1], F32, tag="bzt")
            nc.sync.dma_start(bzt[:], BZ[:])
            bnt = const_pool.tile([128, 1], F32, tag="bnt")
            nc.sync.dma_start(bnt[:], BN[:])

            def load_xnm(eng=None):
                eng = eng or nc.sync
                ts = []
                for jp in range(njp):
                    t = xnm_pool.tile([128, 2, BFX], F8, name="xnmt", tag="xnm")
                    eng.dma_start(t[:], XNM[jp])
                    ts.append(t)
                return ts

            def load_hnm(SRC, eng=None):
                eng = eng or nc.sync
                ts = []
                for jp in range(njp):
                    t = hnm_pool.tile([128, 2, BFH], F8, name="hnmt", tag="hnm")
                    eng.dma_start(t[:], SRC[jp])
                    ts.append(t)
                return ts

            def hop(src_x, src_h, WT, transpose_out=False):
                """One diffusion hop; x part optional.
                Returns (fx, fh, nxs, nhs): fm output tiles and (if
                transpose_out) the re-transposed nm tiles for the next hop.
                c-major: one full jt-accumulation run per output slice, then
                drain that PSUM bank -- only ~3 banks live at a time, so the
                next run never stalls on bank recycling. Chain re-entry
                transposes are folded in per-ibk so the next hop never waits
                on a bulk transpose pass."""
                with_x = src_x is not None
                fx = ([xfm_pool.tile([128, N], F8, name="xfmt", tag="xfm")
                       for _ in range(NXT)] if with_x else None)
                fh = [hfm_pool.tile([128, N], F8, name="hfmt", tag="hfm")
                      for _ in range(NHT)]
                nxs = nhs = None
                if transpose_out:
                    if with_x:
                        nxs = [xnm_pool.tile([128, 2, BFX], F8, name="xnmt",
                                             tag="xnm") for _ in range(njp)]
                    nhs = [hnm_pool.tile([128, 2, BFH], F8, name="hnmt",
                                         tag="hnm") for _ in range(njp)]

                def transpose_blk(ibk):
                    wid = (BFX if with_x else 0) + BFH
                    for q in range(4):
                        it = 4 * ibk + q
                        jp, pl = divmod(it, 2)
                        its = slice(128 * it, 128 * (it + 1))
                        # fp8 transpose-mode PSUM writes need element step 2
                        pt = ps_pool.tile([128, wid, 2], F8,
                                          name="pst", tag="ps")
                        off = 0
                        if with_x:
                            for c in range(NXT):
                                nc.tensor.transpose(
                                    pt[:, 128 * c:128 * (c + 1), 0],
                                    fx[c][:, its], ident[:])
                            off = BFX
                        for c in range(NHT):
                            nc.tensor.transpose(
                                pt[:, off + 128 * c:off + 128 * (c + 1), 0],
                                fh[c][:, its], ident[:])
                        if with_x:
                            nc.vector.tensor_copy(nxs[jp][:, pl, :],
                                                  pt[:, 0:BFX, 0])
                        nc.vector.tensor_copy(nhs[jp][:, pl, :],
                                              pt[:, off:off + BFH, 0])

                # W in [128,2,1024] half-row tiles: each (jp, c) stationary
                # feeds TWO 512-col matmuls, so the 256-col LDWEIGHTS (~213ns,
                # no FWL in DoubleRow mode) hides behind ~214ns of streaming.
                nhh = nbk // 2
                for hh in range(nhh):
                    hbs = slice(1024 * hh, 1024 * (hh + 1))
                    wts = []
                    for jp in range(njp):
                        wt = wt_pool.tile([128, 2, 1024], F8, name="wtt",
                                          tag="wt")
                        nc.sync.dma_start(wt[:], WT[jp][:, :, hbs])
                        wts.append(wt)
                    runs = []
                    if with_x:
                        runs += [(src_x, c, fx[c]) for c in range(NXT)]
                    runs += [(src_h, c, fh[c]) for c in range(NHT)]
                    first_run = True
                    for src, c, fm in runs:
                        p0 = ps_pool.tile([128, 512], F32, name="pst", tag="ps")
                        p1 = ps_pool.tile([128, 512], F32, name="pst", tag="ps")
                        for jp in range(njp):
                            st = src[jp][:, :, 128 * c:128 * (c + 1)]
                            nc.tensor.matmul(
                                p0[:], st, wts[jp][:, :, 0:512],
                                start=(jp == 0), stop=(jp == njp - 1),
                                perf_mode=DR)
                            nc.tensor.matmul(
                                p1[:], st, wts[jp][:, :, 512:1024],
                                start=(jp == 0), stop=(jp == njp - 1),
                                perf_mode=DR)
                        nc.vector.tensor_scalar_mul(
                            fm[:, 1024 * hh:1024 * hh + 512], p0[:], 1.0 / SW)
                        nc.vector.tensor_scalar_mul(
                            fm[:, 1024 * hh + 512:1024 * (hh + 1)], p1[:],
                            1.0 / SW)
                        # previous half's transposes go after this half's
                        # first run so the PE never waits on fm-copy drains.
                        if transpose_out and hh > 0 and first_run:
                            for ib in (2 * hh - 2, 2 * hh - 1):
                                transpose_blk(ib)
                            first_run = False
                if transpose_out:
                    transpose_blk(nbk - 2)
                    transpose_blk(nbk - 1)
                return fx, fh, nxs, nhs

            def mlp_feed(fx, fh, kidx, gates, first):
                """gates: list of (WXbf, WHbf, WX8, WH8, acc_tiles).
                acc_tiles: 2*C SBUF tiles [128, NH] holding SPS-scaled
                logit partial sums; every feed accumulates in SBUF (no
                DRAM round-trip). kidx==0 feeds are bf16 at true scale
                (scaled up by SPS on write); kidx>=1 feeds are fp8 with
                SPS-scaled PSUM."""
                NHl = N // 2
                for WXB, WHB, WX8, WH8, acc in gates:
                    if kidx == 0:
                        wx = wxi_pool.tile([128, 256], MMDT, name="wxt", tag="wxi")
                        nc.scalar.dma_start(wx[:], WXB[:])
                        wh = wxi_pool.tile([128, 128], MMDT, name="wht", tag="wxi")
                        nc.scalar.dma_start(wh[:], WHB[:])
                    else:
                        wx = wxi_pool.tile([128, 256], F8, name="wxt", tag="wxi")
                        nc.scalar.dma_start(wx[:], WX8[kidx - 1])
                        wh = wxi_pool.tile([128, 128], F8, name="wht", tag="wxi")
                        nc.scalar.dma_start(wh[:], WH8[kidx - 1])
                    # b-outer: each batch's logits finish (and downstream
                    # gate work can start) before the next batch's matmuls.
                    w = min(NHl, 512)
                    for b in range(C):
                        pss = [psx_pool.tile([128, 512], F32, name="psxt",
                                             tag="psx") for _ in range(nbk)]
                        for nb in range(nbk):
                            nc.tensor.matmul(
                                pss[nb][:], wh[:],
                                fh[b][:, 512 * nb:512 * (nb + 1)],
                                start=True, stop=False)
                        wxs = wx[:, 128 * (b % 2):128 * (b % 2 + 1)]
                        for nb in range(nbk):
                            nc.tensor.matmul(
                                pss[nb][:], wxs,
                                fx[b // 2][:, 512 * nb:512 * (nb + 1)],
                                start=False, stop=True)
                        for nb in range(nbk):
                            for sub in range(max(1, 512 // NHl)):
                                col = 512 * nb + w * sub
                                pt = acc[2 * b + col // NHl]
                                off = col % NHl
                                pv = pss[nb][:, w * sub:w * (sub + 1)]
                                if first:
                                    nc.vector.tensor_scalar_mul(
                                        pt[:, off:off + w], pv, SPS)
                                else:
                                    nc.vector.tensor_add(
                                        pt[:, off:off + w], pv,
                                        pt[:, off:off + w])

            def load_xfm_spill(kidx):
                ts = []
                for c in range(NXT):
                    t = xfm_pool.tile([128, N], F8, name="xfmt", tag="xfm")
                    nc.scalar.dma_start(t[:], XFMS[kidx - 1][c])
                    ts.append(t)
                return ts

            # ---------------- diffusion 1 (r, z gates) ----------------
            NH = N // 2
            racc = [acc_pool.tile([128, NH], F32, name="acct", tag="acc")
                    for _ in range(2 * C)]
            zacc = [acc_pool.tile([128, NH], F32, name="acct", tag="acc")
                    for _ in range(2 * C)]
            gates1 = [(WRX, WRH, WRX8, WRH8, racc), (WZX, WZH, WZX8, WZH8, zacc)]
            xfm0 = []
            for c in range(NXT):
                t = xfmb_pool.tile([128, N], MMDT, name="xfmbt", tag="xfmb")
                nc.scalar.dma_start(t[:], XFM0[c])
                xfm0.append(t)
            hfm0 = []
            for c in range(NHT):
                t = hfmb_pool.tile([128, N], MMDT, name="hfmbt", tag="hfmb")
                nc.scalar.dma_start(t[:], HFM0[c])
                hfm0.append(t)

            pending = (xfm0, hfm0, 0, gates1, True)
            cur_x, cur_h = load_xnm(), load_hnm(HNM, nc.scalar)
            nxt_nm = None
            for wdir, WT in ((0, WFT), (1, WBT)):
                if wdir == 1:
                    cur_x, cur_h = nxt_nm
                for k in range(1, NHOPS + 1):
                    if k == NHOPS and wdir == 0:
                        # prefetch the backward-chain inputs under this hop
                        nxt_nm = (load_xnm(nc.scalar), load_hnm(HNM, nc.scalar))
                    fx, fh, cur_x, cur_h = hop(cur_x, cur_h, WT,
                                               transpose_out=(k < NHOPS))
                    kidx = wdir * NHOPS + k
                    for c in range(NXT):
                        nc.scalar.dma_start(XFMS[kidx - 1][c], fx[c][:])
                    if pending is not None:
                        mlp_feed(*pending)
                    pending = (fx, fh, kidx, gates1, False)
            mlp_feed(*pending)
            # z logits are final here -- run the sigmoids now, under
            # diffusion 2's compute, so the tail only runs tanh.
            for i in range(2 * C):
                nc.scalar.activation(zacc[i][:], zacc[i][:], AF.Sigmoid,
                                     bias=bzt[:], scale=1.0 / SPS)

            # ------------- gates r, z; assemble rh (nm + fm) -------------
            rh_fm = []
            rh_nm = [hnm_pool.tile([128, 2, BFH], F8, name="hnmt", tag="hnm")
                     for _ in range(njp)]
            for b in range(C):
                rh = hfmb_pool.tile([128, N], MMDT, name="hfmbt", tag="hfmb")
                rh8 = hfm_pool.tile([128, N], F8, name="hfmt", tag="hfm")
                for hx in range(2):
                    hs = slice(NH * hx, NH * (hx + 1))
                    accr = racc[2 * b + hx]
                    nc.scalar.activation(accr[:], accr[:], AF.Sigmoid,
                                         bias=brt[:], scale=1.0 / SPS)
                    h = gate_pool.tile([128, NH], F32, name="gatet", tag="gate")
                    nc.sync.dma_start(h[:], HFMF[b][:, hs])
                    nc.vector.tensor_mul(rh[:, hs], accr[:], h[:])
                    nc.vector.tensor_scalar_mul(rh8[:, hs], rh[:, hs], SA)
                rh_fm.append(rh)
                # rh columns of rh_nm (PE transpose 128-blocks, fp8 planes)
                for g in range(nt // 4):
                    ps = psx_pool.tile([128, 512, 2], F8, name="psxt", tag="psx")
                    for q in range(4):
                        it = 4 * g + q
                        nc.tensor.transpose(
                            ps[:, 128 * q:128 * (q + 1), 0],
                            rh8[:, 128 * it:128 * (it + 1)], ident[:])
                    for q in range(4):
                        it = 4 * g + q
                        jp, pl = divmod(it, 2)
                        nc.vector.tensor_copy(
                            rh_nm[jp][:, pl, b * DH:(b + 1) * DH],
                            ps[:, 128 * q:128 * (q + 1), 0])
            # spill rh_nm for the backward-chain reload
            for jp in range(njp):
                nc.scalar.dma_start(RHNM[jp], rh_nm[jp][:])

            # ---------------- diffusion 2 (n gate) ----------------
            nacc = [acc_pool.tile([128, NH], F32, name="acct", tag="acc")
                    for _ in range(2 * C)]
            gates2 = [(WNX, WNH, WNX8, WNH8, nacc)]
            xfm0b = []
            for c in range(NXT):
                t = xfmb_pool.tile([128, N], MMDT, name="xfmbt", tag="xfmb")
                nc.scalar.dma_start(t[:], XFM0[c])
                xfm0b.append(t)
            pending = (xfm0b, rh_fm, 0, gates2, True)
            cur_h = rh_nm
            nxt_h = None
            for wdir, WT in ((0, WFT), (1, WBT)):
                if wdir == 1:
                    cur_h = nxt_h
                for k in range(1, NHOPS + 1):
                    if k == NHOPS and wdir == 0:
                        nxt_h = load_hnm(RHNM, nc.scalar)
                    _, fh, _, cur_h = hop(None, cur_h, WT,
                                          transpose_out=(k < NHOPS))
                    kidx = wdir * NHOPS + k
                    if pending is not None:
                        mlp_feed(*pending)
                    pending = (load_xfm_spill(kidx), fh, kidx, gates2, False)
            mlp_feed(*pending)

            # ---------------- final gate ----------------
            # out = h + z*(tanh(acc_n) - h); z and n logits live in SBUF,
            # chunked + in-place so chunks pipeline across ACT / DVE / DMA.
            for b in range(C):
                for hx in range(2):
                    hs = slice(NH * hx, NH * (hx + 1))
                    accn = nacc[2 * b + hx]
                    nc.scalar.activation(accn[:], accn[:], AF.Tanh,
                                         bias=bnt[:], scale=1.0 / SPS)
                    h = gate_pool.tile([128, NH], F32, name="gatet", tag="gate")
                    nc.sync.dma_start(h[:], HFMF[b][:, hs])
                    accz = zacc[2 * b + hx]
                    nc.vector.tensor_sub(accn[:], accn[:], h[:])
                    nc.vector.tensor_mul(accz[:], accz[:], accn[:])
                    nc.vector.tensor_add(h[:], accz[:], h[:])
                    nc.sync.dma_start(OUT[b][:, hs], h[:])

    nc.compile()
    return nc


def _pack_gate(W):
    """[128, 7*192] torch-Linear weight -> bf16 k=0 pack (WXB [128,256]
    zero-padded even/odd, WHB [128,128]) + fp8 SW-scaled packs for k=1..6
    (WX8 [6,128,256], WH8 [6,128,128])."""
    f8 = np.dtype(ml_dtypes.float8_e4m3)
    bf = np.dtype(ml_dtypes.bfloat16)
    def xpack(k):
        xs = W[:, k * 192:k * 192 + DX].T          # [64,128]
        out = np.zeros((128, 256), np.float32)
        out[0:64, 0:128] = xs
        out[64:128, 128:256] = xs
        return out
    def hpack(k):
        return W[:, k * 192 + DX:(k + 1) * 192].T  # [128,128]
    wxb = np.ascontiguousarray(xpack(0)).astype(bf)
    whb = np.ascontiguousarray(hpack(0)).astype(bf)
    wx8 = np.ascontiguousarray(np.clip(
        np.stack([xpack(k) * SW_H for k in range(1, 7)]), -240, 240)).astype(f8)
    wh8 = np.ascontiguousarray(np.clip(
        np.stack([hpack(k) * SW_H for k in range(1, 7)]), -240, 240)).astype(f8)
    return wxb, whb, wx8, wh8


SW_H = 512.0
SA_H = 32.0


def _dr_pack(nm_flat, width):
    """[N, width] node-major -> [N//256, 128, 2, width] DoubleRow pairs."""
    N = nm_flat.shape[0]
    t = nm_flat.reshape(N // 128, 128, width)           # [nt, p, w]
    t = t.reshape(N // 256, 2, 128, width).transpose(0, 2, 1, 3)
    return np.ascontiguousarray(t)


_NC_CACHE = {}


def _get_nc(nt):
    if nt not in _NC_CACHE:
        _NC_CACHE[nt] = build_nc(nt)
    return _NC_CACHE[nt]


def make_in_maps(x, h_prev, W_fwd, W_bwd, Wr, br, Wz, bz, Wn, bn):
    f8 = np.dtype(ml_dtypes.float8_e4m3)
    mdt = np.dtype(ml_dtypes.bfloat16)
    x = np.asarray(x, np.float32)
    h_prev = np.asarray(h_prev, np.float32)
    B, N, Din = x.shape
    nt = N // 128
    WfT = np.ascontiguousarray(np.asarray(W_fwd, np.float32).T)   # [N, N] (j, i)
    WbT = np.ascontiguousarray(np.asarray(W_bwd, np.float32).T)
    WfT8 = np.clip(_dr_pack(WfT * SW_H, N), -240, 240).astype(f8)
    WbT8 = np.clip(_dr_pack(WbT * SW_H, N), -240, 240).astype(f8)
    wrxb, wrhb, wrx8, wrh8 = _pack_gate(np.asarray(Wr, np.float32))
    wzxb, wzhb, wzx8, wzh8 = _pack_gate(np.asarray(Wz, np.float32))
    wnxb, wnhb, wnx8, wnh8 = _pack_gate(np.asarray(Wn, np.float32))
    ident = np.ascontiguousarray(np.eye(128, dtype=np.float32)).astype(f8)
    packs = dict(
        wrx=wrxb, wrh=wrhb, wrx8=wrx8, wrh8=wrh8,
        wzx=wzxb, wzh=wzhb, wzx8=wzx8, wzh8=wzh8,
        wnx=wnxb, wnh=wnhb, wnx8=wnx8, wnh8=wnh8)
    brc = np.ascontiguousarray(np.asarray(br, np.float32).reshape(128, 1))
    bzc = np.ascontiguousarray(np.asarray(bz, np.float32).reshape(128, 1))
    bnc = np.ascontiguousarray(np.asarray(bn, np.float32).reshape(128, 1))
    ncores = B // C
    in_maps = []
    for cix in range(ncores):
        xs = x[C * cix:C * (cix + 1)]          # [C,N,64]
        hs = h_prev[C * cix:C * (cix + 1)]     # [C,N,128]
        x_nm = np.ascontiguousarray(xs.transpose(1, 0, 2).reshape(N, BFX))
        h_nm = np.ascontiguousarray(hs.transpose(1, 0, 2).reshape(N, BFH))
        x_nm8 = np.clip(_dr_pack(x_nm * SA_H, BFX), -240, 240).astype(f8)
        h_nm8 = np.clip(_dr_pack(h_nm * SA_H, BFH), -240, 240).astype(f8)
        x_fm0 = np.ascontiguousarray(
            xs.transpose(0, 2, 1).reshape(BFX, N)).reshape(2, 128, N)
        h_fmf = np.ascontiguousarray(hs.transpose(0, 2, 1))  # [C,128,N]
        h_fm0 = h_fmf.reshape(4, 128, N)
        in_maps.append(dict(
            x_nm=x_nm8, h_nm=h_nm8,
            x_fm0=x_fm0.astype(mdt), h_fm0=h_fm0.astype(mdt),
            h_fmf=h_fmf,
            wfT=WfT8, wbT=WbT8, **packs,
            br_c=brc, bz_c=bzc, bn_c=bnc, ident=ident))
    return in_maps, nt, ncores


def kernel(x, h_prev, W_fwd, W_bwd, Wr, br, Wz, bz, Wn, bn, _trace=False):
    in_maps, nt, ncores = make_in_maps(
        x, h_prev, W_fwd, W_bwd, Wr, br, Wz, bz, Wn, bn)
    nc = _get_nc(nt)
    res = run_bass_kernel_spmd(nc, in_maps, list(range(ncores)), trace=_trace)
    outs = [np.ascontiguousarray(res.results[c]["out_fm"].transpose(0, 2, 1))
            for c in range(ncores)]
    full = np.concatenate(outs, axis=0).astype(np.float32)
    if _trace:
        return full, res
    return full
